# revision 15
# baseline (speedup 1.0000x reference)
"""Causal linear attention (elu+1 feature map) on 8 Trainium2 NeuronCores.

Problem: B=2, L=2048, D=512, H=8, dh=64.
    Q/K/V = x @ W_{q,k,v};  Qf/Kf = elu(QK)+1  (Kf, V masked by ~pad)
    out_t = (sum_{s<=t} (Qf_t . Kf_s) V_s) / (Qf_t . sum_{s<=t} Kf_s + eps)
    y = concat_heads(out) @ W_o.T

Sharding: core c handles batch b = c//4 and head pair hp = c%4 (heads
2hp, 2hp+1).  Each core computes its 2 heads' attention over the full
sequence and a partial output projection through the matching 128-column
slice of W_o; partials of the 4 cores of each batch are summed during
unshard.

Layout note: every matmul operand must sit at SBUF base partition 0
(mixed-base matmul sequences lock up the machine), so all per-head
transposed tensors are stored heads-along-columns: qf/kf are (64, 2*L)
with head h in columns [h*L, (h+1)*L), the state is (64, 2*65), O^T is
(64, 2*L).

Device algorithm (per core, all f32, chunked linear attention, chunk=128):
  - x^T staged with D on partitions; QT/KT projections per head
    (M=64 matmuls into a 2-bank fat PSUM tile), phi(x) =
    exp(min(x,0)) + relu(x) fused on ACT+DVE; V in normal layout;
    Kf normal derived from KfT by PE transpose.
  - per chunk i / head h:
      A^T   = Kf_i Qf_i^T                    (PE, dh=64 contraction)
      maskA = A^T * triu                     (DVE; causal within chunk)
      O^T   = V_i^T maskA + S^T Qf_i^T       (PE, PSUM accumulate)
      den   = maskA^T 1 + Qf_i sk            (PE, N=1 matmuls -> columns
                                              batched per 2-chunk group)
      S    += Kf_i^T [V_i | 1]               (PE; running (64 x 65) state
                                              per head, col 64 = Kf cumsum)
    1/(den+eps) computed on 128-token-partition columns (one reciprocal
    per group), applied per head as a per-partition scale at the output
    projection: out_tile = (O_h1^T W_o,h1) * r1 + (O_h0^T W_o,h0) * r0
    via ACT-copy-with-scale + DVE scalar_tensor_tensor.
"""

import sys

for _p in ("/opt/trn_rl_repo", "/opt/pypackages"):
    if _p not in sys.path:
        sys.path.append(_p)

import numpy as np

B, L, D, H, DH = 2, 2048, 512, 8, 64
N_CORES = 8
EPS = 1e-6
P = 128
C = 128                 # attention chunk (tokens)
NCH = L // C            # 16 chunks
GRP = 2                 # chunks per A/den group (4 chunk-heads)
NLC = 4                 # projection L-chunks of 512
VSTR = 130              # V tile stride per token tile: (64 V + 1 one) x 2 heads

_CACHE = {}


def _build(apply_mask: bool):
    import concourse.bacc as bacc
    import concourse.mybir as mybir
    import concourse.tile as tile

    f32 = mybir.dt.float32
    Alu = mybir.AluOpType
    Act = mybir.ActivationFunctionType

    nc = bacc.Bacc("TRN2", target_bir_lowering=False, debug=False,
                   num_devices=N_CORES)

    xT_d = nc.dram_tensor("xT", [4, P, L], f32, kind="ExternalInput").ap()
    wq_d = nc.dram_tensor("wq", [P, 512], f32, kind="ExternalInput").ap()
    wk_d = nc.dram_tensor("wk", [P, 512], f32, kind="ExternalInput").ap()
    wv_d = nc.dram_tensor("wv", [P, 512], f32, kind="ExternalInput").ap()
    wo_d = nc.dram_tensor("wo", [64, 1024], f32, kind="ExternalInput").ap()
    msk_d = nc.dram_tensor("msk", [P, 512], f32, kind="ExternalInput").ap()
    idn_d = nc.dram_tensor("idn", [64, 64], f32, kind="ExternalInput").ap()
    if apply_mask:
        mc_d = nc.dram_tensor("mcol", [P, NCH], f32, kind="ExternalInput").ap()
    part_d = nc.dram_tensor("part", [L, 512], f32, kind="ExternalOutput").ap()

    with tile.TileContext(nc) as tc:
        with tc.tile_pool(name="persist", bufs=1) as pp:
            xt = [pp.tile([P, L], f32, tag=f"xt{d}", name=f"xt{d}")
                  for d in range(4)]
            wq = pp.tile([P, 512], f32, tag="wq")
            wk = pp.tile([P, 512], f32, tag="wk")
            wv = pp.tile([P, 512], f32, tag="wv")
            wo = pp.tile([64, 1024], f32, tag="wo")
            msk = pp.tile([P, 512], f32, tag="msk")
            idn = pp.tile([64, 64], f32, tag="idn")
            qf = pp.tile([64, 2 * L], f32, tag="qf")     # head h at cols h*L+
            kf = pp.tile([64, 2 * L], f32, tag="kf")
            vn = pp.tile([P, NCH * VSTR], f32, tag="vn")
            kn = pp.tile([P, L], f32, tag="kn")          # tile i: i*128+64h+e
            ot = pp.tile([64, 2 * L], f32, tag="ot")
            s_sb = pp.tile([64, 130], f32, tag="s_sb")   # head h at cols h*65+
            if apply_mask:
                mc = pp.tile([P, NCH], f32, tag="mc")

            for d in range(4):
                nc.sync.dma_start(out=xt[d][:], in_=xT_d[d])
            nc.sync.dma_start(out=wq[:], in_=wq_d[:])
            nc.sync.dma_start(out=wk[:], in_=wk_d[:])
            nc.sync.dma_start(out=wv[:], in_=wv_d[:])
            nc.sync.dma_start(out=wo[:], in_=wo_d[:])
            nc.sync.dma_start(out=msk[:], in_=msk_d[:])
            nc.sync.dma_start(out=idn[:], in_=idn_d[:])
            if apply_mask:
                nc.sync.dma_start(out=mc[:], in_=mc_d[:])

            # vn viewed as [P, tt, h, 65]; col 64 of each (tt,h) block = 1.0
            vn4 = vn[:].rearrange("p (t h c) -> p t h c", t=NCH, h=2, c=65)
            nc.vector.memset(vn4[:, :, :, 64:65], 1.0)
            nc.vector.memset(s_sb[:], 0.0)
            qf2 = qf[:].rearrange("p (h t) -> p h t", h=2)
            kf2 = kf[:].rearrange("p (h t) -> p h t", h=2)

            # ---------------- phase 1: projections ----------------
            with tc.tile_pool(name="pj", bufs=2, space="PSUM") as pjp, \
                 tc.tile_pool(name="vps", bufs=2, space="PSUM") as vpp, \
                 tc.tile_pool(name="trp", bufs=2, space="PSUM") as trp, \
                 tc.tile_pool(name="w1", bufs=2) as w1:
                for lc in range(NLC):
                    sl = slice(lc * 512, (lc + 1) * 512)
                    for which, w_t, dve_lean in (("q", wq, True),
                                                 ("k", wk, False)):
                        # fat (64, 1024) PSUM tile, head h at cols h*512
                        ps = pjp.tile([64, 1024], f32, tag="pj",
                                      name=f"{which}ps")
                        for h in (0, 1):
                            for d in range(4):
                                nc.tensor.matmul(
                                    ps[:, h * 512:(h + 1) * 512],
                                    lhsT=w_t[:, d * P + 64 * h:
                                             d * P + 64 * h + 64],
                                    rhs=xt[d][:, sl],
                                    start=(d == 0), stop=(d == 3))
                        ps3 = ps[:].rearrange("p (h t) -> p h t", h=2)
                        dst = (qf2 if which == "q" else kf2)[:, :, sl]
                        mt = w1.tile([64, 1024], f32, tag="m")
                        if dve_lean:
                            nc.vector.tensor_scalar_min(mt[:], ps[:], 0.0)
                            et = w1.tile([64, 1024], f32, tag="e")
                            nc.scalar.activation(et[:], mt[:], Act.Exp)
                        else:
                            nc.scalar.activation(mt[:], ps[:], Act.Relu,
                                                 scale=-1.0)
                            et = w1.tile([64, 1024], f32, tag="e")
                            nc.scalar.activation(et[:], mt[:], Act.Exp,
                                                 scale=-1.0)
                        et3 = et[:].rearrange("p (h t) -> p h t", h=2)
                        nc.vector.scalar_tensor_tensor(
                            dst, ps3, 0.0, et3, op0=Alu.max, op1=Alu.add)
                    # V projection (normal layout) + copy into vn
                    v_ps = vpp.tile([P, 512], f32, tag="v")
                    for k in range(4):
                        ti = lc * 4 + k
                        for d in range(4):
                            nc.tensor.matmul(
                                v_ps[:, k * P:(k + 1) * P],
                                lhsT=xt[d][:, ti * P:(ti + 1) * P],
                                rhs=wv[:, d * P:(d + 1) * P],
                                start=(d == 0), stop=(d == 3))
                    v_src = v_ps[:].rearrange("p (k h e) -> p k h e", k=4, h=2)
                    if apply_mask:
                        for k in range(4):
                            ti = lc * 4 + k
                            nc.vector.tensor_scalar_mul(
                                vn4[:, ti, :, 0:64], v_src[:, k],
                                mc[:, ti:ti + 1])
                    else:
                        nc.vector.tensor_copy(
                            vn4[:, lc * 4:(lc + 1) * 4, :, 0:64], v_src)
                    # Kf normal layout via PE transpose of KfT
                    tr_ps = trp.tile([P, 512], f32, tag="tr")
                    for k in range(4):
                        ti = lc * 4 + k
                        for h in (0, 1):
                            nc.tensor.transpose(
                                tr_ps[:, k * P + 64 * h:k * P + 64 * h + 64],
                                kf2[:, h, ti * P:(ti + 1) * P], idn[:])
                    if apply_mask:
                        for k in range(4):
                            ti = lc * 4 + k
                            nc.vector.tensor_scalar_mul(
                                kn[:, ti * P:(ti + 1) * P],
                                tr_ps[:, k * P:(k + 1) * P],
                                mc[:, ti:ti + 1])
                    else:
                        nc.vector.tensor_copy(kn[:, sl], tr_ps[:])

            # ------------- phase 2+3: attention + out-projection -------------
            with tc.tile_pool(name="aps", bufs=2, space="PSUM") as app, \
                 tc.tile_pool(name="ops", bufs=2, space="PSUM") as opp, \
                 tc.tile_pool(name="prj", bufs=2, space="PSUM") as prp, \
                 tc.tile_pool(name="upd", bufs=1, space="PSUM") as updp, \
                 tc.tile_pool(name="den", bufs=1, space="PSUM") as denp, \
                 tc.tile_pool(name="w2", bufs=2) as w2:
                ot2 = ot[:].rearrange("p (h t) -> p h t", h=2)
                for g in range(NCH // GRP):
                    chunks = [g * GRP + u for u in range(GRP)]
                    jhs = [(i, h) for i in chunks for h in (0, 1)]
                    # A^T for the group's 4 chunk-heads
                    a_ps = app.tile([P, 512], f32, tag="a")
                    for j, (i, h) in enumerate(jhs):
                        cs = slice(i * C, (i + 1) * C)
                        nc.tensor.matmul(
                            a_ps[:, j * P:(j + 1) * P],
                            lhsT=kf2[:, h, cs], rhs=qf2[:, h, cs],
                            start=True, stop=True)
                    am = w2.tile([P, 512], f32, tag="am")
                    nc.vector.tensor_mul(am[:], a_ps[:], msk[:])
                    if apply_mask:
                        for u, i in enumerate(chunks):
                            nc.vector.tensor_scalar_mul(
                                am[:, u * 256:(u + 1) * 256],
                                am[:, u * 256:(u + 1) * 256],
                                mc[:, i:i + 1])
                    den_ps = denp.tile([P, 2 * GRP], f32, tag="den")
                    o_tiles = [opp.tile([64, 256], f32, tag="o", name=f"o{i}")
                               for i in chunks]
                    upd_tiles = [updp.tile([64, 130], f32, tag="upd",
                                           name=f"u{i}") for i in chunks]
                    for u, i in enumerate(chunks):
                        o_ps = o_tiles[u]
                        cs = slice(i * C, (i + 1) * C)
                        for h in (0, 1):
                            j = 2 * u + h
                            os_ = slice(h * C, (h + 1) * C)
                            ss = slice(h * 65, h * 65 + 64)
                            vh = vn[:, i * VSTR + 65 * h:i * VSTR + 65 * h + 64]
                            vh1 = vn[:, i * VSTR + 65 * h:i * VSTR + 65 * h + 65]
                            ones_col = vn[:, i * VSTR + 64:i * VSTR + 65]
                            # O^T = [inter] + intra
                            if i > 0:
                                nc.tensor.matmul(
                                    o_ps[:, os_], lhsT=s_sb[:, ss],
                                    rhs=qf2[:, h, cs], start=True, stop=False)
                            nc.tensor.matmul(
                                o_ps[:, os_], lhsT=vh,
                                rhs=am[:, j * P:(j + 1) * P],
                                start=(i == 0), stop=True)
                            # den column
                            nc.tensor.matmul(
                                den_ps[:, j:j + 1],
                                lhsT=am[:, j * P:(j + 1) * P],
                                rhs=ones_col, start=True, stop=(i == 0))
                            if i > 0:
                                nc.tensor.matmul(
                                    den_ps[:, j:j + 1], lhsT=qf2[:, h, cs],
                                    rhs=s_sb[:, h * 65 + 64:h * 65 + 65],
                                    start=False, stop=True)
                            # state update for this chunk-head
                            nc.tensor.matmul(
                                upd_tiles[u][:, h * 65:(h + 1) * 65],
                                lhsT=kn[:, i * P + 64 * h:i * P + 64 * h + 64],
                                rhs=vh1, start=True, stop=True)
                        # fold this chunk's update into the running state
                        # (next chunk's inter/den-inter depend on it)
                        nc.vector.tensor_add(s_sb[:], s_sb[:], upd_tiles[u][:])
                    # batched 1/(den+eps): columns over the 128 token-partitions
                    dsb = w2.tile([P, 2 * GRP], f32, tag="dsb")
                    nc.vector.tensor_scalar_add(dsb[:], den_ps[:], EPS)
                    rcl = w2.tile([P, 2 * GRP], f32, tag="rcl")
                    nc.vector.reciprocal(rcl[:], dsb[:])
                    # O^T psum -> sbuf
                    for u, i in enumerate(chunks):
                        cs = slice(i * C, (i + 1) * C)
                        src = o_tiles[u][:].rearrange("p (h t) -> p h t", h=2)
                        nc.vector.tensor_copy(ot2[:, :, cs], src)
                    # out-projection per token tile, division via row scales
                    for u, i in enumerate(chunks):
                        cs = slice(i * C, (i + 1) * C)
                        ps0 = prp.tile([P, 512], f32, tag="prj")
                        nc.tensor.matmul(ps0[:], lhsT=ot2[:, 0, cs],
                                         rhs=wo[:, 0:512], start=True,
                                         stop=True)
                        ps1 = prp.tile([P, 512], f32, tag="prj")
                        nc.tensor.matmul(ps1[:], lhsT=ot2[:, 1, cs],
                                         rhs=wo[:, 512:1024], start=True,
                                         stop=True)
                        tsb = w2.tile([P, 512], f32, tag="tsb")
                        nc.scalar.activation(tsb[:], ps0[:], Act.Copy,
                                             scale=rcl[:, 2 * u:2 * u + 1])
                        osb = w2.tile([P, 512], f32, tag="osb")
                        nc.vector.scalar_tensor_tensor(
                            osb[:], ps1[:], rcl[:, 2 * u + 1:2 * u + 2], tsb[:],
                            op0=Alu.mult, op1=Alu.add)
                        nc.sync.dma_start(out=part_d[i * P:(i + 1) * P, :],
                                          in_=osb[:])

    nc.compile()
    return nc


def _get_program(apply_mask: bool):
    key = bool(apply_mask)
    if key not in _CACHE:
        from concourse.bass_interp import get_hw_module
        nc = _build(key)
        nc.m = get_hw_module(nc.m)
        _CACHE[key] = nc
    return _CACHE[key]


def _in_maps(x, key_padding_mask, W_q, W_k, W_v, W_o, apply_mask):
    triu = np.triu(np.ones((P, P), np.float32))
    msk = np.tile(triu, (1, 4)).copy()
    idn = np.eye(64, dtype=np.float32)
    maps = []
    for c in range(N_CORES):
        b, hp = divmod(c, 4)
        xT = np.ascontiguousarray(x[b].T).reshape(4, P, L)

        def wslice(W):
            w = W[:, 2 * hp:2 * hp + 2, :].reshape(D, P)
            return np.ascontiguousarray(
                w.reshape(4, P, P).transpose(1, 0, 2).reshape(P, 512))

        # wo: (64, 1024), head h at cols h*512: wo[e, h*512+j] =
        # W_o[j, 128*hp + 64*h + e]
        wo_s = W_o[:, P * hp:P * (hp + 1)].T.reshape(2, 64, 512)
        wo = np.ascontiguousarray(wo_s.transpose(1, 0, 2).reshape(64, 1024))
        m = {"xT": xT, "wq": wslice(W_q), "wk": wslice(W_k),
             "wv": wslice(W_v), "wo": wo, "msk": msk, "idn": idn}
        if apply_mask:
            keep = (~key_padding_mask[b]).astype(np.float32)  # (L,)
            m["mcol"] = np.ascontiguousarray(keep.reshape(NCH, P).T)
        maps.append(m)
    return maps


def kernel(x, key_padding_mask, W_q, W_k, W_v, W_o, _trace=False):
    from concourse.bass_utils import run_bass_kernel_spmd

    x = np.asarray(x, dtype=np.float32)
    key_padding_mask = np.asarray(key_padding_mask).astype(bool)
    apply_mask = bool(key_padding_mask.any())
    nc = _get_program(apply_mask)
    maps = _in_maps(x, key_padding_mask, np.asarray(W_q, np.float32),
                    np.asarray(W_k, np.float32), np.asarray(W_v, np.float32),
                    np.asarray(W_o, np.float32), apply_mask)
    res = run_bass_kernel_spmd(nc, maps, core_ids=list(range(N_CORES)),
                               trace=_trace)
    kernel.last_results = res
    out = np.zeros((B, L, D), np.float32)
    for c in range(N_CORES):
        out[c // 4] += res.results[c]["part"]
    return out


# revision 17
# speedup vs baseline: 1.1593x; 1.1593x over previous
"""Causal linear attention (elu+1 feature map) on 8 Trainium2 NeuronCores.

Problem: B=2, L=2048, D=512, H=8, dh=64.
    Q/K/V = x @ W_{q,k,v};  Qf/Kf = elu(QK)+1  (Kf, V masked by ~pad)
    out_t = (sum_{s<=t} (Qf_t . Kf_s) V_s) / (Qf_t . sum_{s<=t} Kf_s + eps)
    y = concat_heads(out) @ W_o.T

Sharding: core c handles batch b = c//4 and head pair hp = c%4 (heads
2hp, 2hp+1).  Each core computes its 2 heads' attention over the full
sequence and a partial output projection through the matching 128-column
slice of W_o; partials of the 4 cores of each batch are summed during
unshard.

Layout note: every matmul operand must sit at SBUF base partition 0
(mixed-base matmul sequences lock up the machine), so all per-head
transposed tensors are stored heads-along-columns: qf/kf are (64, 2*L)
with head h in columns [h*L, (h+1)*L), the state is (64, 2*65), O^T is
(64, 2*L).

Device algorithm (per core, all f32, chunked linear attention, chunk=128):
  - x^T staged with D on partitions; QT/KT projections per head
    (M=64 matmuls into a 2-bank fat PSUM tile), phi(x) =
    exp(min(x,0)) + relu(x) fused on ACT+DVE; V in normal layout;
    Kf normal derived from KfT by PE transpose.
  - per chunk i / head h:
      A^T   = Kf_i Qf_i^T                    (PE, dh=64 contraction)
      maskA = A^T * triu                     (DVE; causal within chunk)
      O^T   = V_i^T maskA + S^T Qf_i^T       (PE, PSUM accumulate)
      den   = maskA^T 1 + Qf_i sk            (PE, N=1 matmuls -> columns
                                              batched per 2-chunk group)
      S    += Kf_i^T [V_i | 1]               (PE; running (64 x 65) state
                                              per head, col 64 = Kf cumsum)
    1/(den+eps) computed on 128-token-partition columns (one reciprocal
    per group), applied per head as a per-partition scale at the output
    projection: out_tile = (O_h1^T W_o,h1) * r1 + (O_h0^T W_o,h0) * r0
    via ACT-copy-with-scale + DVE scalar_tensor_tensor.
"""

import sys

for _p in ("/opt/trn_rl_repo", "/opt/pypackages"):
    if _p not in sys.path:
        sys.path.append(_p)

import numpy as np

B, L, D, H, DH = 2, 2048, 512, 8, 64
N_CORES = 8
EPS = 1e-6
P = 128
C = 128                 # attention chunk (tokens)
NCH = L // C            # 16 chunks
GRP = 2                 # chunks per A/den group (4 chunk-heads)
NLC = 4                 # projection L-chunks of 512
VSTR = 130              # V tile stride per token tile: (64 V + 1 one) x 2 heads

_CACHE = {}


def _build(apply_mask: bool):
    import concourse.bacc as bacc
    import concourse.mybir as mybir
    import concourse.tile as tile

    f32 = mybir.dt.float32
    Alu = mybir.AluOpType
    Act = mybir.ActivationFunctionType

    nc = bacc.Bacc("TRN2", target_bir_lowering=False, debug=False,
                   num_devices=N_CORES)

    xT_d = nc.dram_tensor("xT", [4, P, L], f32, kind="ExternalInput").ap()
    wq_d = nc.dram_tensor("wq", [P, 512], f32, kind="ExternalInput").ap()
    wk_d = nc.dram_tensor("wk", [P, 512], f32, kind="ExternalInput").ap()
    wv_d = nc.dram_tensor("wv", [P, 512], f32, kind="ExternalInput").ap()
    wo_d = nc.dram_tensor("wo", [64, 1024], f32, kind="ExternalInput").ap()
    msk_d = nc.dram_tensor("msk", [P, 512], f32, kind="ExternalInput").ap()
    idn_d = nc.dram_tensor("idn", [64, 64], f32, kind="ExternalInput").ap()
    if apply_mask:
        mc_d = nc.dram_tensor("mcol", [P, NCH], f32, kind="ExternalInput").ap()
    part_d = nc.dram_tensor("part", [L, 512], f32, kind="ExternalOutput").ap()

    with tile.TileContext(nc) as tc:
        with tc.tile_pool(name="persist", bufs=1) as pp:
            xt = [pp.tile([P, L], f32, tag=f"xt{d}", name=f"xt{d}")
                  for d in range(4)]
            wq = pp.tile([P, 512], f32, tag="wq")
            wk = pp.tile([P, 512], f32, tag="wk")
            wv = pp.tile([P, 512], f32, tag="wv")
            wo = pp.tile([64, 1024], f32, tag="wo")
            msk = pp.tile([P, 512], f32, tag="msk")
            idn = pp.tile([64, 64], f32, tag="idn")
            qf = pp.tile([64, 2 * L], f32, tag="qf")     # head h at cols h*L+
            kf = pp.tile([64, 2 * L], f32, tag="kf")
            vn = pp.tile([P, NCH * VSTR], f32, tag="vn")
            kn = pp.tile([P, L], f32, tag="kn")          # tile i: i*128+64h+e
            ot = pp.tile([64, 2 * L], f32, tag="ot")
            s_sb = pp.tile([64, 130], f32, tag="s_sb")   # head h at cols h*65+
            if apply_mask:
                mc = pp.tile([P, NCH], f32, tag="mc")

            for lc in range(NLC):
                for d in range(4):
                    nc.sync.dma_start(
                        out=xt[d][:, lc * 512:(lc + 1) * 512],
                        in_=xT_d[d, :, lc * 512:(lc + 1) * 512])
            nc.sync.dma_start(out=wq[:], in_=wq_d[:])
            nc.sync.dma_start(out=wk[:], in_=wk_d[:])
            nc.sync.dma_start(out=wv[:], in_=wv_d[:])
            nc.sync.dma_start(out=wo[:], in_=wo_d[:])
            nc.sync.dma_start(out=msk[:], in_=msk_d[:])
            nc.sync.dma_start(out=idn[:], in_=idn_d[:])
            if apply_mask:
                nc.sync.dma_start(out=mc[:], in_=mc_d[:])

            # vn viewed as [P, tt, h, 65]; col 64 of each (tt,h) block = 1.0
            vn4 = vn[:].rearrange("p (t h c) -> p t h c", t=NCH, h=2, c=65)
            nc.vector.memset(vn4[:, :, :, 64:65], 1.0)
            nc.vector.memset(s_sb[:], 0.0)
            qf2 = qf[:].rearrange("p (h t) -> p h t", h=2)
            kf2 = kf[:].rearrange("p (h t) -> p h t", h=2)

            # ---------------- phase 1: projections ----------------
            with tc.tile_pool(name="pj", bufs=2, space="PSUM") as pjp, \
                 tc.tile_pool(name="vps", bufs=2, space="PSUM") as vpp, \
                 tc.tile_pool(name="trp", bufs=2, space="PSUM") as trp, \
                 tc.tile_pool(name="w1", bufs=2) as w1:
                for lc in range(NLC):
                    sl = slice(lc * 512, (lc + 1) * 512)
                    for which, w_t, dve_lean in (("q", wq, True),
                                                 ("k", wk, False)):
                        # fat (64, 1024) PSUM tile, head h at cols h*512
                        ps = pjp.tile([64, 1024], f32, tag="pj",
                                      name=f"{which}ps")
                        for h in (0, 1):
                            for d in range(4):
                                nc.tensor.matmul(
                                    ps[:, h * 512:(h + 1) * 512],
                                    lhsT=w_t[:, d * P + 64 * h:
                                             d * P + 64 * h + 64],
                                    rhs=xt[d][:, sl],
                                    start=(d == 0), stop=(d == 3))
                        ps3 = ps[:].rearrange("p (h t) -> p h t", h=2)
                        dst = (qf2 if which == "q" else kf2)[:, :, sl]
                        mt = w1.tile([64, 1024], f32, tag="m")
                        if dve_lean:
                            nc.vector.tensor_scalar_min(mt[:], ps[:], 0.0)
                            et = w1.tile([64, 1024], f32, tag="e")
                            nc.scalar.activation(et[:], mt[:], Act.Exp)
                        else:
                            nc.scalar.activation(mt[:], ps[:], Act.Relu,
                                                 scale=-1.0)
                            et = w1.tile([64, 1024], f32, tag="e")
                            nc.scalar.activation(et[:], mt[:], Act.Exp,
                                                 scale=-1.0)
                        et3 = et[:].rearrange("p (h t) -> p h t", h=2)
                        nc.vector.scalar_tensor_tensor(
                            dst, ps3, 0.0, et3, op0=Alu.max, op1=Alu.add)
                    # V projection (normal layout) + copy into vn
                    v_ps = vpp.tile([P, 512], f32, tag="v")
                    for k in range(4):
                        ti = lc * 4 + k
                        for d in range(4):
                            nc.tensor.matmul(
                                v_ps[:, k * P:(k + 1) * P],
                                lhsT=xt[d][:, ti * P:(ti + 1) * P],
                                rhs=wv[:, d * P:(d + 1) * P],
                                start=(d == 0), stop=(d == 3))
                    v_src = v_ps[:].rearrange("p (k h e) -> p k h e", k=4, h=2)
                    if apply_mask:
                        for k in range(4):
                            ti = lc * 4 + k
                            nc.vector.tensor_scalar_mul(
                                vn4[:, ti, :, 0:64], v_src[:, k],
                                mc[:, ti:ti + 1])
                    else:
                        nc.vector.tensor_copy(
                            vn4[:, lc * 4:(lc + 1) * 4, :, 0:64], v_src)
                    # Kf normal layout via PE transpose of KfT
                    tr_ps = trp.tile([P, 512], f32, tag="tr")
                    for k in range(4):
                        ti = lc * 4 + k
                        for h in (0, 1):
                            nc.tensor.transpose(
                                tr_ps[:, k * P + 64 * h:k * P + 64 * h + 64],
                                kf2[:, h, ti * P:(ti + 1) * P], idn[:])
                    if apply_mask:
                        for k in range(4):
                            ti = lc * 4 + k
                            nc.vector.tensor_scalar_mul(
                                kn[:, ti * P:(ti + 1) * P],
                                tr_ps[:, k * P:(k + 1) * P],
                                mc[:, ti:ti + 1])
                    else:
                        nc.vector.tensor_copy(kn[:, sl], tr_ps[:])

            # ------------- phase 2+3: attention + out-projection -------------
            with tc.tile_pool(name="aps", bufs=1, space="PSUM") as app, \
                 tc.tile_pool(name="ops", bufs=2, space="PSUM") as opp, \
                 tc.tile_pool(name="prj", bufs=3, space="PSUM") as prp, \
                 tc.tile_pool(name="upd", bufs=2, space="PSUM") as updp, \
                 tc.tile_pool(name="w2", bufs=2) as w2:
                ot2 = ot[:].rearrange("p (h t) -> p h t", h=2)
                for g in range(NCH // GRP):
                    chunks = [g * GRP + u for u in range(GRP)]
                    jhs = [(i, h) for i in chunks for h in (0, 1)]
                    # A^T for the group's 4 chunk-heads
                    a_ps = app.tile([P, 512], f32, tag="a")
                    for j, (i, h) in enumerate(jhs):
                        cs = slice(i * C, (i + 1) * C)
                        nc.tensor.matmul(
                            a_ps[:, j * P:(j + 1) * P],
                            lhsT=kf2[:, h, cs], rhs=qf2[:, h, cs],
                            start=True, stop=True)
                    am = w2.tile([P, 512], f32, tag="am")
                    nc.vector.tensor_mul(am[:], a_ps[:], msk[:])
                    if apply_mask:
                        for u, i in enumerate(chunks):
                            nc.vector.tensor_scalar_mul(
                                am[:, u * 256:(u + 1) * 256],
                                am[:, u * 256:(u + 1) * 256],
                                mc[:, i:i + 1])
                    den_ps = updp.tile([P, 2 * GRP], f32, tag="upd",
                                       name="den", padded_shape=[P, 130])
                    o_tiles = [opp.tile([64, 256], f32, tag="o", name=f"o{i}")
                               for i in chunks]
                    upd_tiles = [updp.tile([64, 130], f32, tag="upd",
                                           name=f"u{i}") for i in chunks]
                    for u, i in enumerate(chunks):
                        o_ps = o_tiles[u]
                        cs = slice(i * C, (i + 1) * C)
                        for h in (0, 1):
                            j = 2 * u + h
                            os_ = slice(h * C, (h + 1) * C)
                            ss = slice(h * 65, h * 65 + 64)
                            vh = vn[:, i * VSTR + 65 * h:i * VSTR + 65 * h + 64]
                            vh1 = vn[:, i * VSTR + 65 * h:i * VSTR + 65 * h + 65]
                            ones_col = vn[:, i * VSTR + 64:i * VSTR + 65]
                            # O^T = [inter] + intra
                            if i > 0:
                                nc.tensor.matmul(
                                    o_ps[:, os_], lhsT=s_sb[:, ss],
                                    rhs=qf2[:, h, cs], start=True, stop=False)
                            nc.tensor.matmul(
                                o_ps[:, os_], lhsT=vh,
                                rhs=am[:, j * P:(j + 1) * P],
                                start=(i == 0), stop=True)
                            # den column
                            nc.tensor.matmul(
                                den_ps[:, j:j + 1],
                                lhsT=am[:, j * P:(j + 1) * P],
                                rhs=ones_col, start=True, stop=(i == 0))
                            if i > 0:
                                nc.tensor.matmul(
                                    den_ps[:, j:j + 1], lhsT=qf2[:, h, cs],
                                    rhs=s_sb[:, h * 65 + 64:h * 65 + 65],
                                    start=False, stop=True)
                            # state update for this chunk-head
                            nc.tensor.matmul(
                                upd_tiles[u][:, h * 65:(h + 1) * 65],
                                lhsT=kn[:, i * P + 64 * h:i * P + 64 * h + 64],
                                rhs=vh1, start=True, stop=True)
                        # fold this chunk's update into the running state
                        # (next chunk's inter/den-inter depend on it)
                        nc.vector.tensor_add(s_sb[:], s_sb[:], upd_tiles[u][:])
                    # batched 1/(den+eps): columns over the 128 token-partitions
                    dsb = w2.tile([P, 2 * GRP], f32, tag="dsb")
                    nc.vector.tensor_scalar_add(dsb[:], den_ps[:], EPS)
                    rcl = w2.tile([P, 2 * GRP], f32, tag="rcl")
                    nc.vector.reciprocal(rcl[:], dsb[:])
                    # O^T psum -> sbuf
                    for u, i in enumerate(chunks):
                        cs = slice(i * C, (i + 1) * C)
                        src = o_tiles[u][:].rearrange("p (h t) -> p h t", h=2)
                        nc.vector.tensor_copy(ot2[:, :, cs], src)
                    # out-projection per token tile, division via row scales
                    for u, i in enumerate(chunks):
                        cs = slice(i * C, (i + 1) * C)
                        ps0 = prp.tile([P, 512], f32, tag="prj")
                        nc.tensor.matmul(ps0[:], lhsT=ot2[:, 0, cs],
                                         rhs=wo[:, 0:512], start=True,
                                         stop=True)
                        ps1 = prp.tile([P, 512], f32, tag="prj")
                        nc.tensor.matmul(ps1[:], lhsT=ot2[:, 1, cs],
                                         rhs=wo[:, 512:1024], start=True,
                                         stop=True)
                        tsb = w2.tile([P, 512], f32, tag="tsb")
                        nc.scalar.activation(tsb[:], ps0[:], Act.Copy,
                                             scale=rcl[:, 2 * u:2 * u + 1])
                        osb = w2.tile([P, 512], f32, tag="osb")
                        nc.vector.scalar_tensor_tensor(
                            osb[:], ps1[:], rcl[:, 2 * u + 1:2 * u + 2], tsb[:],
                            op0=Alu.mult, op1=Alu.add)
                        nc.sync.dma_start(out=part_d[i * P:(i + 1) * P, :],
                                          in_=osb[:])

    nc.compile()
    return nc


def _get_program(apply_mask: bool):
    key = bool(apply_mask)
    if key not in _CACHE:
        from concourse.bass_interp import get_hw_module
        nc = _build(key)
        nc.m = get_hw_module(nc.m)
        _CACHE[key] = nc
    return _CACHE[key]


def _in_maps(x, key_padding_mask, W_q, W_k, W_v, W_o, apply_mask):
    triu = np.triu(np.ones((P, P), np.float32))
    msk = np.tile(triu, (1, 4)).copy()
    idn = np.eye(64, dtype=np.float32)
    maps = []
    for c in range(N_CORES):
        b, hp = divmod(c, 4)
        xT = np.ascontiguousarray(x[b].T).reshape(4, P, L)

        def wslice(W):
            w = W[:, 2 * hp:2 * hp + 2, :].reshape(D, P)
            return np.ascontiguousarray(
                w.reshape(4, P, P).transpose(1, 0, 2).reshape(P, 512))

        # wo: (64, 1024), head h at cols h*512: wo[e, h*512+j] =
        # W_o[j, 128*hp + 64*h + e]
        wo_s = W_o[:, P * hp:P * (hp + 1)].T.reshape(2, 64, 512)
        wo = np.ascontiguousarray(wo_s.transpose(1, 0, 2).reshape(64, 1024))
        m = {"xT": xT, "wq": wslice(W_q), "wk": wslice(W_k),
             "wv": wslice(W_v), "wo": wo, "msk": msk, "idn": idn}
        if apply_mask:
            keep = (~key_padding_mask[b]).astype(np.float32)  # (L,)
            m["mcol"] = np.ascontiguousarray(keep.reshape(NCH, P).T)
        maps.append(m)
    return maps


def kernel(x, key_padding_mask, W_q, W_k, W_v, W_o, _trace=False):
    from concourse.bass_utils import run_bass_kernel_spmd

    x = np.asarray(x, dtype=np.float32)
    key_padding_mask = np.asarray(key_padding_mask).astype(bool)
    apply_mask = bool(key_padding_mask.any())
    nc = _get_program(apply_mask)
    maps = _in_maps(x, key_padding_mask, np.asarray(W_q, np.float32),
                    np.asarray(W_k, np.float32), np.asarray(W_v, np.float32),
                    np.asarray(W_o, np.float32), apply_mask)
    res = run_bass_kernel_spmd(nc, maps, core_ids=list(range(N_CORES)),
                               trace=_trace)
    kernel.last_results = res
    out = np.zeros((B, L, D), np.float32)
    for c in range(N_CORES):
        out[c // 4] += res.results[c]["part"]
    return out
